# revision 46
# baseline (speedup 1.0000x reference)
# Multi-head attention (B=4, S=2048, D=1024, H=16) on 8 NeuronCores.
#
# Sharding: batch x head-group. Core c handles batch b=c//2 and heads
# 8*(c%2) .. 8*(c%2)+7 (a 512-wide slice of the model dim). Each core
# computes QKV projections for its slice, causal attention for its 8
# heads, and a row-parallel partial of the output projection. The host
# sums the two partials per batch and adds bo.
#
# QKV/scores matmuls run as float32r (full-rate PE mode, fp32 storage);
# the attention value path (probs, V, context, Wo) runs in bf16 with
# fp32 PSUM accumulation. Attention is computed transposed
# (scoresT[k, q]) so the probs feed the AV matmul and the context feeds
# the output projection with no on-chip transposes. Softmax skips the
# max-subtraction pass (scores are small: |s*scale| < ~2.5). Each
# head's V block carries 64 ones-columns so the same AV matmul also
# produces the softmax denominators replicated on PSUM partitions
# 64..127; 1/sum is computed as exp(-ln(sum)) on the scalar engine
# (the custom fast-reciprocal DVE op does not compile under this
# container's walrus). Causality is exploited at three granularities:
# fully-masked k-blocks are skipped, partially-masked diagonal blocks
# trim the AV matmul to the valid column range, and the in-block
# triangle is zeroed with one [128,128] upper-tri mask multiply.
#
# Emission order interleaves the three phases (QKV t-chunk i ->
# attention q-chunk for all heads once its causal k-range is resident
# -> output projection at the end) so the PE-heavy projections overlap
# the ACT-heavy softmax throughout; the cost-model timeline sim shows
# PE 95-100% / ACT ~95% occupancy through the body (~280us/core).

import sys

for _p in ("/opt/trn_rl_repo", "/root/.axon_site/_ro/trn_rl_repo"):
    if _p not in sys.path:
        sys.path.append(_p)

import ml_dtypes
import numpy as np

import concourse.bass as bass
import concourse.mybir as mybir
import concourse.tile as tile
from concourse.bass_utils import run_bass_kernel_spmd
from concourse.masks import make_upper_triangular

B, S, D, H = 4, 2048, 1024, 16
HD = D // H            # 64
N_CORES = 8
GH = 8                 # heads per core
C = GH * HD            # 512 local model dims per core
SCALE = HD ** -0.5
F32 = mybir.dt.float32
F32R = mybir.dt.float32r
BF16 = mybir.dt.bfloat16

T_CHUNK = 256          # t-tile for QKV projections
QC = 512               # q columns per attention chunk
KB = 128               # k rows per attention block
N_KB = S // KB         # 16
N_QC = S // QC         # 4
EXP_GROUP = 2          # k-blocks per batched exp (2 psum banks x 2 bufs)


def _split_multi_waits(nc):
    """walrus in this container accepts only one sync-wait per instruction.
    Hoist all but the last wait of any multi-wait instruction onto NoOps
    inserted just before it on the same engine (sequencers execute their
    queue in order, so chained single waits are equivalent)."""
    for f in nc.m.functions:
        for blk in f.blocks:
            new_insts = []
            for inst in blk.instructions:
                si = inst.sync_info
                if si is not None and si.on_wait and len(si.on_wait) > 1:
                    waits = list(si.on_wait)
                    for i, w in enumerate(waits[:-1]):
                        nop = mybir.InstNoOp(name=f"{inst.name}_sw{i}", ins=[], outs=[])
                        nop.engine = inst.engine
                        nop.sync_info = mybir.SyncInfo(on_wait=[w], on_update=[])
                        new_insts.append(nop)
                    si.on_wait = [waits[-1]]
                new_insts.append(inst)
            blk.instructions[:] = new_insts


def _emit_kernel(nc, reps=1):
    xt = nc.dram_tensor("xt", [D, S], F32R, kind="ExternalInput").ap()
    wqt = nc.dram_tensor("wqt", [D, C], F32R, kind="ExternalInput").ap()
    wkt = nc.dram_tensor("wkt", [D, C], F32R, kind="ExternalInput").ap()
    wvt = nc.dram_tensor("wvt", [D, C], F32R, kind="ExternalInput").ap()
    bqv = nc.dram_tensor("bqv", [C], F32, kind="ExternalInput").ap()
    bkv = nc.dram_tensor("bkv", [C], F32, kind="ExternalInput").ap()
    bvb = nc.dram_tensor("bvb", [128, C], F32, kind="ExternalInput").ap()
    wot = nc.dram_tensor("wot", [C, D], BF16, kind="ExternalInput").ap()
    out = nc.dram_tensor("out", [S, D], F32, kind="ExternalOutput").ap()

    NFC = D // 128      # 8 f-chunks of the projection contraction
    NCC = C // 128      # 4 c-chunks of the local model dim

    with tile.TileContext(nc) as tc:
        import contextlib

        ctx = contextlib.ExitStack()
        with ctx:
            consts = ctx.enter_context(tc.tile_pool(name="consts", bufs=1))
            wpool = ctx.enter_context(tc.tile_pool(name="wpool", bufs=1))
            qkv = ctx.enter_context(tc.tile_pool(name="qkv", bufs=1))
            xtp = ctx.enter_context(tc.tile_pool(name="xtp", bufs=2))
            expp = ctx.enter_context(tc.tile_pool(name="expp", bufs=4))
            ctxp = ctx.enter_context(tc.tile_pool(name="ctxp", bufs=1))
            smallp = ctx.enter_context(tc.tile_pool(name="smallp", bufs=1))
            outp = ctx.enter_context(tc.tile_pool(name="outp", bufs=3))

            ps_qkv = ctx.enter_context(
                tc.tile_pool(name="ps_qkv", bufs=2, space="PSUM")
            )
            ps_sc = ctx.enter_context(
                tc.tile_pool(name="ps_sc", bufs=2, space="PSUM")
            )
            ps_av = ctx.enter_context(
                tc.tile_pool(name="ps_av", bufs=2, space="PSUM")
            )


            # ---- constants -------------------------------------------------
            tri = consts.tile([128, 128], BF16)      # tri[p, c] = 1.0 iff p <= c
            make_upper_triangular(nc, tri[:, :], val=1.0, diag=True)

            bv_bc = consts.tile([128, C], F32)      # bv broadcast across partitions
            nc.scalar.dma_start(out=bv_bc[:, :], in_=bvb)

            bq_sb = consts.tile([128, NCC], F32)    # bq[cc*128 + p] at [p, cc]
            nc.gpsimd.dma_start(out=bq_sb[:, :], in_=bqv.rearrange("(cc p) -> p cc", p=128))
            bk_sb = consts.tile([128, NCC], F32)
            nc.gpsimd.dma_start(out=bk_sb[:, :], in_=bkv.rearrange("(cc p) -> p cc", p=128))

            # ---- xt prefetch (chunks 0..1 ahead of the weight queue) ------
            _xt_pref = {}
            for _tci in range(min(2, S // T_CHUNK)):
                xt_c = xtp.tile([128, NFC, T_CHUNK], F32R, name="xt_c")
                nc.sync.dma_start(
                    out=xt_c[:, :, :],
                    in_=xt.rearrange("(fc p) t -> p fc t", p=128)[
                        :, :, _tci * T_CHUNK : (_tci + 1) * T_CHUNK
                    ],
                )
                _xt_pref[_tci] = xt_c

            # ---- weights ---------------------------------------------------
            wq_sb = wpool.tile([128, NFC, C], F32R)
            wk_sb = wpool.tile([128, NFC, C], F32R)
            wv_sb = wpool.tile([128, NFC, C], F32R)
            for fc in range(NFC):
                nc.gpsimd.dma_start(out=wq_sb[:, fc, :], in_=wqt[fc * 128 : (fc + 1) * 128, :])
            for fc in range(NFC):
                nc.scalar.dma_start(out=wk_sb[:, fc, :], in_=wkt[fc * 128 : (fc + 1) * 128, :])
            for fc in range(NFC):
                nc.scalar.dma_start(out=wv_sb[:, fc, :], in_=wvt[fc * 128 : (fc + 1) * 128, :])
            wo_sb = wpool.tile([128, NCC, D], BF16)
            for cc in range(NCC):
                nc.scalar.dma_start(out=wo_sb[:, cc, :], in_=wot[cc * 128 : (cc + 1) * 128, :])

            # ---- persistent activations -----------------------------------
            qt_sb = qkv.tile([128, NCC, S], F32R)    # qT: [c within chunk, cc, t]
            kt_sb = qkv.tile([128, NCC, S], F32R)
            v_sb = qkv.tile([128, N_KB, GH, 2 * HD], BF16)  # v + 64 ones cols
            ctx_sb = ctxp.tile([128, NCC, S], BF16)  # ctxT: [c within chunk, cc, q]

            nc.gpsimd.memset(v_sb[:, :, :, HD : 2 * HD], 1.0)

            _phases = "123"

            def emit_phase1(tci):
                t0 = tci * T_CHUNK
                if tci in _xt_pref:
                    xt_c = _xt_pref.pop(tci)
                else:
                    xt_c = xtp.tile([128, NFC, T_CHUNK], F32R, name="xt_c")
                    nc.sync.dma_start(
                        out=xt_c[:, :, :],
                        in_=xt.rearrange("(fc p) t -> p fc t", p=128)[:, :, t0 : t0 + T_CHUNK],
                    )
                for name, w_sb, b_sb, y_sb in (
                    ("q", wq_sb, bq_sb, qt_sb),
                    ("k", wk_sb, bk_sb, kt_sb),
                ):
                    for cc in range(NCC):
                        ps = ps_qkv.tile([128, T_CHUNK], F32, name=f"ps_{name}", tag="ps_qkv")
                        for fc in range(NFC):
                            nc.tensor.matmul(
                                ps[:, :],
                                w_sb[:, fc, cc * 128 : (cc + 1) * 128],
                                xt_c[:, fc, :],
                                start=(fc == 0),
                                stop=(fc == NFC - 1),
                            )
                        # yT = ps + bias (per-partition scalar), PSUM -> SBUF
                        nc.vector.tensor_scalar_add(
                            y_sb[:, cc, t0 : t0 + T_CHUNK], ps[:, :], b_sb[:, cc : cc + 1]
                        )
                for tt in range(T_CHUNK // 128):
                    kb = (t0 + tt * 128) // KB
                    ps = ps_qkv.tile([128, C], F32, name="ps_v", tag="ps_qkv")
                    for fc in range(NFC):
                        nc.tensor.matmul(
                            ps[:, :],
                            xt_c[:, fc, tt * 128 : (tt + 1) * 128],
                            wv_sb[:, fc, :],
                            start=(fc == 0),
                            stop=(fc == NFC - 1),
                        )
                    nc.vector.tensor_add(
                        v_sb[:, kb, :, 0:HD],
                        ps.rearrange("p (h d) -> p h d", h=GH),
                        bv_bc.rearrange("p (h d) -> p h d", h=GH),
                    )

            def emit_phase2(h, qi):
                    hc = h // 2             # which 128-partition chunk of qt/kt
                    hp = 64 * (h % 2)       # partition offset inside the chunk
                    q0 = qi * QC
                    nkb = 4 * qi + 4        # causal: k-blocks 0 .. 4qi+3
                    av_ps = ps_av.tile([128, QC], F32)
                    n_grp = (nkb + EXP_GROUP - 1) // EXP_GROUP
                    for gi in range(n_grp):
                        kb_lo = gi * EXP_GROUP
                        kb_hi = min(kb_lo + EXP_GROUP, nkb)
                        gw = kb_hi - kb_lo
                        sc_ps = ps_sc.tile([128, EXP_GROUP, QC], F32)
                        for kb in range(kb_lo, kb_hi):
                            nc.tensor.matmul(
                                sc_ps[:, kb - kb_lo, :],
                                kt_sb[hp : hp + HD, hc, kb * KB : (kb + 1) * KB],
                                qt_sb[hp : hp + HD, hc, q0 : q0 + QC],
                                start=True,
                                stop=True,
                            )
                        et = expp.tile([128, EXP_GROUP, QC], BF16)
                        nc.scalar.activation(
                            et[:, 0:gw, :].rearrange("p g q -> p (g q)"),
                            sc_ps[:, 0:gw, :].rearrange("p g q -> p (g q)"),
                            mybir.ActivationFunctionType.Exp,
                            bias=0.0,
                            scale=SCALE,
                        )
                        for kb in range(kb_lo, kb_hi):
                            m = kb - 4 * qi  # >= 0 on the causal diagonal
                            off = 128 * m if m >= 0 else 0
                            if m >= 0:
                                nc.vector.tensor_mul(
                                    et[:, kb - kb_lo, off : off + 128],
                                    et[:, kb - kb_lo, off : off + 128],
                                    tri[:, :],
                                )
                            nc.tensor.matmul(
                                av_ps[:, off:QC],
                                v_sb[:, kb, h, :],
                                et[:, kb - kb_lo, off:QC],
                                start=(kb == 0),
                                stop=(kb == nkb - 1),
                            )
                    lsum = smallp.tile([HD, QC], F32)
                    nc.scalar.activation(
                        lsum[:, :], av_ps[HD : 2 * HD, :],
                        mybir.ActivationFunctionType.Ln,
                    )
                    rbc = smallp.tile([HD, QC], F32)
                    nc.scalar.activation(
                        rbc[:, :], lsum[:, :],
                        mybir.ActivationFunctionType.Exp,
                        scale=-1.0,
                    )
                    nc.vector.tensor_mul(
                        ctx_sb[hp : hp + HD, hc, q0 : q0 + QC],
                        av_ps[0:HD, :],
                        rbc[:, :],
                    )

            def emit_phase3(qq):
                for eh in range(2):
                    ps = ps_qkv.tile([128, D // 2], F32, name="ps_op", tag="ps_qkv")
                    for cc in range(NCC):
                        nc.tensor.matmul(
                            ps[:, :],
                            ctx_sb[:, cc, qq * 128 : (qq + 1) * 128],
                            wo_sb[:, cc, eh * (D // 2) : (eh + 1) * (D // 2)],
                            start=(cc == 0),
                            stop=(cc == NCC - 1),
                        )
                    o_sb = outp.tile([128, D // 2], F32, name="o_sb")
                    nc.vector.tensor_copy(o_sb[:, :], ps[:, :])
                    nc.sync.dma_start(
                        out=out[qq * 128 : (qq + 1) * 128, eh * (D // 2) : (eh + 1) * (D // 2)],
                        in_=o_sb[:, :],
                    )

            TPQ = QC // T_CHUNK  # t-chunks per attention q-chunk
            for _rep in range(reps):
                for tci in range(S // T_CHUNK):
                    if "1" in _phases:
                        emit_phase1(tci)
                    if (tci + 1) % TPQ == 0:
                        qi = tci // TPQ
                        if "2" in _phases:
                            for h in range(GH):
                                emit_phase2(h, qi)
                if "3" in _phases:
                    for qq in range(S // 128):
                        emit_phase3(qq)

    _split_multi_waits(nc)
    return nc


_CACHED = {}


def _build(reps=1):
    if reps not in _CACHED:
        nc = bass.Bass("TRN2", target_bir_lowering=False, debug=False)
        _CACHED[reps] = _emit_kernel(nc, reps)
    return _CACHED[reps]


def _reference_numpy(x, Wq, bq, Wk, bk, Wv, bv, Wo, bo, attention_mask):
    """Fallback for non-all-ones attention masks (spec fills ones)."""
    scale = HD ** -0.5
    out = np.empty((B, S, D), np.float32)
    causal = np.triu(np.ones((S, S), bool), k=1)
    for b in range(B):
        q = (x[b] @ Wq.T + bq).reshape(S, H, HD).transpose(1, 0, 2)
        k = (x[b] @ Wk.T + bk).reshape(S, H, HD).transpose(1, 0, 2)
        v = (x[b] @ Wv.T + bv).reshape(S, H, HD).transpose(1, 0, 2)
        o = np.empty((H, S, HD), np.float32)
        pad = (attention_mask[b] == 0)[None, :]
        for h in range(H):
            s = (q[h] @ k[h].T) * scale
            s[causal] = -np.inf
            s = np.where(pad, np.float32(-1e9), s)
            s -= s.max(-1, keepdims=True)
            e = np.exp(s)
            p = e / e.sum(-1, keepdims=True)
            o[h] = p @ v[h]
        ctx = o.transpose(1, 0, 2).reshape(S, D)
        out[b] = ctx @ Wo.T + bo
    return out


def kernel(x, Wq, bq, Wk, bk, Wv, bv, Wo, bo, attention_mask):
    x = np.asarray(x, np.float32)
    Wq, bq = np.asarray(Wq, np.float32), np.asarray(bq, np.float32)
    Wk, bk = np.asarray(Wk, np.float32), np.asarray(bk, np.float32)
    Wv, bv = np.asarray(Wv, np.float32), np.asarray(bv, np.float32)
    Wo, bo = np.asarray(Wo, np.float32), np.asarray(bo, np.float32)
    attention_mask = np.asarray(attention_mask)

    if not np.all(attention_mask == 1):
        return _reference_numpy(x, Wq, bq, Wk, bk, Wv, bv, Wo, bo, attention_mask)

    nc = _build()

    xts = [np.ascontiguousarray(x[b].T) for b in range(B)]
    shards = []
    for g in range(2):
        cs = slice(g * C, (g + 1) * C)
        shards.append(
            dict(
                wqt=np.ascontiguousarray(Wq[cs, :].T),
                wkt=np.ascontiguousarray(Wk[cs, :].T),
                wvt=np.ascontiguousarray(Wv[cs, :].T),
                bqv=np.ascontiguousarray(bq[cs]),
                bkv=np.ascontiguousarray(bk[cs]),
                bvb=np.ascontiguousarray(np.broadcast_to(bv[cs], (128, C))),
                wot=np.ascontiguousarray(Wo[:, cs].T).astype(ml_dtypes.bfloat16),
            )
        )
    in_maps = []
    for c in range(N_CORES):
        b, g = c // 2, c % 2
        in_maps.append(dict(xt=xts[b], **shards[g]))

    res = run_bass_kernel_spmd(nc, in_maps, core_ids=list(range(N_CORES)))

    out = np.empty((B, S, D), np.float32)
    for b in range(B):
        out[b] = res.results[2 * b]["out"] + res.results[2 * b + 1]["out"] + bo
    return out


# revision 49
# speedup vs baseline: 1.0030x; 1.0030x over previous
# Multi-head attention (B=4, S=2048, D=1024, H=16) on 8 NeuronCores.
#
# Sharding: batch x head-group. Core c handles batch b=c//2 and heads
# 8*(c%2) .. 8*(c%2)+7 (a 512-wide slice of the model dim). Each core
# computes QKV projections for its slice, causal attention for its 8
# heads, and a row-parallel partial of the output projection. The host
# sums the two partials per batch and adds bo.
#
# QKV/scores matmuls run as float32r (full-rate PE mode, fp32 storage);
# the attention value path (probs, V, context, Wo) runs in bf16 with
# fp32 PSUM accumulation. Attention is computed transposed
# (scoresT[k, q]) so the probs feed the AV matmul and the context feeds
# the output projection with no on-chip transposes. Softmax skips the
# max-subtraction pass (scores are small: |s*scale| < ~2.5). Each
# head's V block carries 64 ones-columns so the same AV matmul also
# produces the softmax denominators replicated on PSUM partitions
# 64..127; 1/sum is computed as exp(-ln(sum)) on the scalar engine
# (the custom fast-reciprocal DVE op does not compile under this
# container's walrus). Causality is exploited at three granularities:
# fully-masked k-blocks are skipped, partially-masked diagonal blocks
# trim the AV matmul to the valid column range, and the in-block
# triangle is zeroed with one [128,128] upper-tri mask multiply.
#
# Emission order interleaves the three phases (QKV t-chunk i ->
# attention q-chunk for all heads once its causal k-range is resident
# -> output projection at the end) so the PE-heavy projections overlap
# the ACT-heavy softmax throughout; the cost-model timeline sim shows
# PE 95-100% / ACT ~95% occupancy through the body (~280us/core).

import sys

for _p in ("/opt/trn_rl_repo", "/root/.axon_site/_ro/trn_rl_repo"):
    if _p not in sys.path:
        sys.path.append(_p)

import ml_dtypes
import numpy as np

import concourse.bass as bass
import concourse.mybir as mybir
import concourse.tile as tile
from concourse.bass_utils import run_bass_kernel_spmd
from concourse.masks import make_upper_triangular

B, S, D, H = 4, 2048, 1024, 16
HD = D // H            # 64
N_CORES = 8
GH = 8                 # heads per core
C = GH * HD            # 512 local model dims per core
SCALE = HD ** -0.5
F32 = mybir.dt.float32
F32R = mybir.dt.float32r
BF16 = mybir.dt.bfloat16

T_CHUNK = 256          # t-tile for QKV projections
QC = 512               # q columns per attention chunk
KB = 128               # k rows per attention block
N_KB = S // KB         # 16
N_QC = S // QC         # 4
EXP_GROUP = 2          # k-blocks per batched exp (2 psum banks x 2 bufs)


def _split_multi_waits(nc):
    """walrus in this container accepts only one sync-wait per instruction.
    Hoist all but the last wait of any multi-wait instruction onto NoOps
    inserted just before it on the same engine (sequencers execute their
    queue in order, so chained single waits are equivalent)."""
    for f in nc.m.functions:
        for blk in f.blocks:
            new_insts = []
            for inst in blk.instructions:
                si = inst.sync_info
                if si is not None and si.on_wait and len(si.on_wait) > 1:
                    waits = list(si.on_wait)
                    for i, w in enumerate(waits[:-1]):
                        nop = mybir.InstNoOp(name=f"{inst.name}_sw{i}", ins=[], outs=[])
                        nop.engine = inst.engine
                        nop.sync_info = mybir.SyncInfo(on_wait=[w], on_update=[])
                        new_insts.append(nop)
                    si.on_wait = [waits[-1]]
                new_insts.append(inst)
            blk.instructions[:] = new_insts


def _emit_kernel(nc, reps=1):
    xt = nc.dram_tensor("xt", [D, S], F32R, kind="ExternalInput").ap()
    wqt = nc.dram_tensor("wqt", [D, C], F32R, kind="ExternalInput").ap()
    wkt = nc.dram_tensor("wkt", [D, C], F32R, kind="ExternalInput").ap()
    wvt = nc.dram_tensor("wvt", [D, C], F32R, kind="ExternalInput").ap()
    bqv = nc.dram_tensor("bqv", [C], F32, kind="ExternalInput").ap()
    bkv = nc.dram_tensor("bkv", [C], F32, kind="ExternalInput").ap()
    bvb = nc.dram_tensor("bvb", [128, C], F32, kind="ExternalInput").ap()
    wot = nc.dram_tensor("wot", [C, D], BF16, kind="ExternalInput").ap()
    out = nc.dram_tensor("out", [S, D], F32, kind="ExternalOutput").ap()

    NFC = D // 128      # 8 f-chunks of the projection contraction
    NCC = C // 128      # 4 c-chunks of the local model dim

    with tile.TileContext(nc) as tc:
        import contextlib

        ctx = contextlib.ExitStack()
        with ctx:
            consts = ctx.enter_context(tc.tile_pool(name="consts", bufs=1))
            wpool = ctx.enter_context(tc.tile_pool(name="wpool", bufs=1))
            qkv = ctx.enter_context(tc.tile_pool(name="qkv", bufs=1))
            xtp = ctx.enter_context(tc.tile_pool(name="xtp", bufs=2))
            expp = ctx.enter_context(tc.tile_pool(name="expp", bufs=4))
            ctxp = ctx.enter_context(tc.tile_pool(name="ctxp", bufs=1))
            smallp = ctx.enter_context(tc.tile_pool(name="smallp", bufs=1))
            outp = ctx.enter_context(tc.tile_pool(name="outp", bufs=3))

            ps_qkv = ctx.enter_context(
                tc.tile_pool(name="ps_qkv", bufs=2, space="PSUM")
            )
            ps_sc = ctx.enter_context(
                tc.tile_pool(name="ps_sc", bufs=2, space="PSUM")
            )
            ps_av = ctx.enter_context(
                tc.tile_pool(name="ps_av", bufs=2, space="PSUM")
            )


            # ---- constants -------------------------------------------------
            tri = consts.tile([128, 128], BF16)      # tri[p, c] = 1.0 iff p <= c
            make_upper_triangular(nc, tri[:, :], val=1.0, diag=True)

            bv_bc = consts.tile([128, C], F32)      # bv broadcast across partitions
            nc.scalar.dma_start(out=bv_bc[:, :], in_=bvb)

            bq_sb = consts.tile([128, NCC], F32)    # bq[cc*128 + p] at [p, cc]
            nc.gpsimd.dma_start(out=bq_sb[:, :], in_=bqv.rearrange("(cc p) -> p cc", p=128))
            bk_sb = consts.tile([128, NCC], F32)
            nc.gpsimd.dma_start(out=bk_sb[:, :], in_=bkv.rearrange("(cc p) -> p cc", p=128))

            # ---- xt prefetch (chunks 0..1 ahead of the weight queue) ------
            _xt_pref = {}
            for _tci in range(min(2, S // T_CHUNK)):
                xt_c = xtp.tile([128, NFC, T_CHUNK], F32R, name="xt_c")
                nc.sync.dma_start(
                    out=xt_c[:, :, :],
                    in_=xt.rearrange("(fc p) t -> p fc t", p=128)[
                        :, :, _tci * T_CHUNK : (_tci + 1) * T_CHUNK
                    ],
                )
                _xt_pref[_tci] = xt_c

            # ---- weights ---------------------------------------------------
            wq_sb = wpool.tile([128, NFC, C], F32R)
            wk_sb = wpool.tile([128, NFC, C], F32R)
            wv_sb = wpool.tile([128, NFC, C], F32R)
            for fc in range(NFC):
                nc.gpsimd.dma_start(out=wq_sb[:, fc, :], in_=wqt[fc * 128 : (fc + 1) * 128, :])
            for fc in range(NFC):
                nc.scalar.dma_start(out=wk_sb[:, fc, :], in_=wkt[fc * 128 : (fc + 1) * 128, :])
            for fc in range(NFC):
                nc.scalar.dma_start(out=wv_sb[:, fc, :], in_=wvt[fc * 128 : (fc + 1) * 128, :])
            wo_sb = wpool.tile([128, NCC, D], BF16)
            for cc in range(NCC):
                nc.scalar.dma_start(out=wo_sb[:, cc, :], in_=wot[cc * 128 : (cc + 1) * 128, :])

            # ---- persistent activations -----------------------------------
            qt_sb = qkv.tile([128, NCC, S], F32R)    # qT: [c within chunk, cc, t]
            kt_sb = qkv.tile([128, NCC, S], F32R)
            v_sb = qkv.tile([128, N_KB, GH, 2 * HD], BF16)  # v + 64 ones cols
            ctx_sb = ctxp.tile([128, NCC, S], BF16)  # ctxT: [c within chunk, cc, q]

            nc.gpsimd.memset(v_sb[:, :, :, HD : 2 * HD], 1.0)

            _phases = "123"

            def emit_phase1(tci):
                t0 = tci * T_CHUNK
                if tci in _xt_pref:
                    xt_c = _xt_pref.pop(tci)
                else:
                    xt_c = xtp.tile([128, NFC, T_CHUNK], F32R, name="xt_c")
                    nc.sync.dma_start(
                        out=xt_c[:, :, :],
                        in_=xt.rearrange("(fc p) t -> p fc t", p=128)[:, :, t0 : t0 + T_CHUNK],
                    )
                for name, w_sb, b_sb, y_sb in (
                    ("q", wq_sb, bq_sb, qt_sb),
                    ("k", wk_sb, bk_sb, kt_sb),
                ):
                    for cc in range(NCC):
                        ps = ps_qkv.tile([128, T_CHUNK], F32, name=f"ps_{name}", tag="ps_qkv")
                        for fc in range(NFC):
                            nc.tensor.matmul(
                                ps[:, :],
                                w_sb[:, fc, cc * 128 : (cc + 1) * 128],
                                xt_c[:, fc, :],
                                start=(fc == 0),
                                stop=(fc == NFC - 1),
                            )
                        # yT = ps + bias (per-partition scalar), PSUM -> SBUF
                        nc.vector.tensor_scalar_add(
                            y_sb[:, cc, t0 : t0 + T_CHUNK], ps[:, :], b_sb[:, cc : cc + 1]
                        )
                for tt in range(T_CHUNK // 128):
                    kb = (t0 + tt * 128) // KB
                    ps = ps_qkv.tile([128, C], F32, name="ps_v", tag="ps_qkv")
                    for fc in range(NFC):
                        nc.tensor.matmul(
                            ps[:, :],
                            xt_c[:, fc, tt * 128 : (tt + 1) * 128],
                            wv_sb[:, fc, :],
                            start=(fc == 0),
                            stop=(fc == NFC - 1),
                        )
                    nc.vector.tensor_add(
                        v_sb[:, kb, :, 0:HD],
                        ps.rearrange("p (h d) -> p h d", h=GH),
                        bv_bc.rearrange("p (h d) -> p h d", h=GH),
                    )

            def emit_phase2(h, qi):
                    hc = h // 2             # which 128-partition chunk of qt/kt
                    hp = 64 * (h % 2)       # partition offset inside the chunk
                    q0 = qi * QC
                    nkb = 4 * qi + 4        # causal: k-blocks 0 .. 4qi+3
                    av_ps = ps_av.tile([128, QC], F32)
                    n_grp = (nkb + EXP_GROUP - 1) // EXP_GROUP
                    for gi in range(n_grp):
                        kb_lo = gi * EXP_GROUP
                        kb_hi = min(kb_lo + EXP_GROUP, nkb)
                        gw = kb_hi - kb_lo
                        sc_ps = ps_sc.tile([128, EXP_GROUP, QC], F32)
                        for kb in range(kb_lo, kb_hi):
                            nc.tensor.matmul(
                                sc_ps[:, kb - kb_lo, :],
                                kt_sb[hp : hp + HD, hc, kb * KB : (kb + 1) * KB],
                                qt_sb[hp : hp + HD, hc, q0 : q0 + QC],
                                start=True,
                                stop=True,
                            )
                        et = expp.tile([128, EXP_GROUP, QC], BF16)
                        # cols < 128*m of diagonal block m are never read by
                        # AV; a rectangular trim to the group's min offset is
                        # safe and cuts ACT work on the causal tail.
                        g_min_m = kb_lo - 4 * qi
                        g_off = 128 * g_min_m if g_min_m > 0 else 0
                        nc.scalar.activation(
                            et[:, 0:gw, g_off:QC],
                            sc_ps[:, 0:gw, g_off:QC],
                            mybir.ActivationFunctionType.Exp,
                            bias=0.0,
                            scale=SCALE,
                        )
                        for kb in range(kb_lo, kb_hi):
                            m = kb - 4 * qi  # >= 0 on the causal diagonal
                            off = 128 * m if m >= 0 else 0
                            if m >= 0:
                                nc.vector.tensor_mul(
                                    et[:, kb - kb_lo, off : off + 128],
                                    et[:, kb - kb_lo, off : off + 128],
                                    tri[:, :],
                                )
                            nc.tensor.matmul(
                                av_ps[:, off:QC],
                                v_sb[:, kb, h, :],
                                et[:, kb - kb_lo, off:QC],
                                start=(kb == 0),
                                stop=(kb == nkb - 1),
                            )
                    lsum = smallp.tile([HD, QC], F32)
                    nc.scalar.activation(
                        lsum[:, :], av_ps[HD : 2 * HD, :],
                        mybir.ActivationFunctionType.Ln,
                    )
                    rbc = smallp.tile([HD, QC], F32)
                    nc.scalar.activation(
                        rbc[:, :], lsum[:, :],
                        mybir.ActivationFunctionType.Exp,
                        scale=-1.0,
                    )
                    nc.vector.tensor_mul(
                        ctx_sb[hp : hp + HD, hc, q0 : q0 + QC],
                        av_ps[0:HD, :],
                        rbc[:, :],
                    )

            def emit_phase3(qq):
                for eh in range(2):
                    ps = ps_qkv.tile([128, D // 2], F32, name="ps_op", tag="ps_qkv")
                    for cc in range(NCC):
                        nc.tensor.matmul(
                            ps[:, :],
                            ctx_sb[:, cc, qq * 128 : (qq + 1) * 128],
                            wo_sb[:, cc, eh * (D // 2) : (eh + 1) * (D // 2)],
                            start=(cc == 0),
                            stop=(cc == NCC - 1),
                        )
                    o_sb = outp.tile([128, D // 2], F32, name="o_sb")
                    nc.vector.tensor_copy(o_sb[:, :], ps[:, :])
                    nc.sync.dma_start(
                        out=out[qq * 128 : (qq + 1) * 128, eh * (D // 2) : (eh + 1) * (D // 2)],
                        in_=o_sb[:, :],
                    )

            TPQ = QC // T_CHUNK  # t-chunks per attention q-chunk
            for _rep in range(reps):
                for tci in range(S // T_CHUNK):
                    if "1" in _phases:
                        emit_phase1(tci)
                    if (tci + 1) % TPQ == 0:
                        qi = tci // TPQ
                        if "2" in _phases:
                            for h in range(GH):
                                emit_phase2(h, qi)
                if "3" in _phases:
                    for qq in range(S // 128):
                        emit_phase3(qq)

    _split_multi_waits(nc)
    return nc


_CACHED = {}


def _build(reps=1):
    if reps not in _CACHED:
        nc = bass.Bass("TRN2", target_bir_lowering=False, debug=False)
        _CACHED[reps] = _emit_kernel(nc, reps)
    return _CACHED[reps]


def _reference_numpy(x, Wq, bq, Wk, bk, Wv, bv, Wo, bo, attention_mask):
    """Fallback for non-all-ones attention masks (spec fills ones)."""
    scale = HD ** -0.5
    out = np.empty((B, S, D), np.float32)
    causal = np.triu(np.ones((S, S), bool), k=1)
    for b in range(B):
        q = (x[b] @ Wq.T + bq).reshape(S, H, HD).transpose(1, 0, 2)
        k = (x[b] @ Wk.T + bk).reshape(S, H, HD).transpose(1, 0, 2)
        v = (x[b] @ Wv.T + bv).reshape(S, H, HD).transpose(1, 0, 2)
        o = np.empty((H, S, HD), np.float32)
        pad = (attention_mask[b] == 0)[None, :]
        for h in range(H):
            s = (q[h] @ k[h].T) * scale
            s[causal] = -np.inf
            s = np.where(pad, np.float32(-1e9), s)
            s -= s.max(-1, keepdims=True)
            e = np.exp(s)
            p = e / e.sum(-1, keepdims=True)
            o[h] = p @ v[h]
        ctx = o.transpose(1, 0, 2).reshape(S, D)
        out[b] = ctx @ Wo.T + bo
    return out


def kernel(x, Wq, bq, Wk, bk, Wv, bv, Wo, bo, attention_mask):
    x = np.asarray(x, np.float32)
    Wq, bq = np.asarray(Wq, np.float32), np.asarray(bq, np.float32)
    Wk, bk = np.asarray(Wk, np.float32), np.asarray(bk, np.float32)
    Wv, bv = np.asarray(Wv, np.float32), np.asarray(bv, np.float32)
    Wo, bo = np.asarray(Wo, np.float32), np.asarray(bo, np.float32)
    attention_mask = np.asarray(attention_mask)

    if not np.all(attention_mask == 1):
        return _reference_numpy(x, Wq, bq, Wk, bk, Wv, bv, Wo, bo, attention_mask)

    nc = _build()

    xts = [np.ascontiguousarray(x[b].T) for b in range(B)]
    shards = []
    for g in range(2):
        cs = slice(g * C, (g + 1) * C)
        shards.append(
            dict(
                wqt=np.ascontiguousarray(Wq[cs, :].T),
                wkt=np.ascontiguousarray(Wk[cs, :].T),
                wvt=np.ascontiguousarray(Wv[cs, :].T),
                bqv=np.ascontiguousarray(bq[cs]),
                bkv=np.ascontiguousarray(bk[cs]),
                bvb=np.ascontiguousarray(np.broadcast_to(bv[cs], (128, C))),
                wot=np.ascontiguousarray(Wo[:, cs].T).astype(ml_dtypes.bfloat16),
            )
        )
    in_maps = []
    for c in range(N_CORES):
        b, g = c // 2, c % 2
        in_maps.append(dict(xt=xts[b], **shards[g]))

    res = run_bass_kernel_spmd(nc, in_maps, core_ids=list(range(N_CORES)))

    out = np.empty((B, S, D), np.float32)
    for b in range(B):
        out[b] = res.results[2 * b]["out"] + res.results[2 * b + 1]["out"] + bo
    return out
